# revision 21
# baseline (speedup 1.0000x reference)
"""Bass/Tile TRN2 kernel for nn_CausalAttention (softmax + tril-matmul renorm).

V3 restructure around the identity  masked @ v == s @ cumsum(v):
    out[i] = (sum_t s[i,t] * PV[t]) / (sum_t (t+1) * s[i,t]),   s = exp(q k^T / sqrt(D))
with PV[t] = prefix-sum of v rows, split (for fp8 precision) as
    PV[t] = PVt_within[t] + CVS[tile(t)]
where PVt_within is the within-128-tile prefix (small magnitudes, fp8 ok) and
CVS carries all cross-tile aggregates in bf16 (tile colsums VS computed in
high precision from x row-sums: VS = xrs @ wv_bf).

Per core (512 q rows):
  local:  xT (transposes, bf16+fp8), kT/qT (fp8 DR), v (fp8 DR, full scale),
          PVt = within-tile prefix of v (triu matmuls), xrs row sums,
          VS = xrsT-chunks @ wv_bf (bf16)
  comm:   AllGather kT in two key-halves (scores start on half 1),
          AllGather [PVt fp8 | VS f32]
  A:      zT tiles [key,q] (fp8 DR), exp -> m0 fp8, per-pair selector matmul
          -> rs (per-tile rowsums of s, rows 0..31) + den (row 64)
  B:      num[q,d] = sum_pairs m0_pair^T mm PV_pair (fp8 DR)
          + rank-32 close: rs^T mm CVS (bf16), CVS = stril32 @ VS_all
          out = num * recip(den) / 64
"""
import numpy as np
from contextlib import ExitStack

import concourse.bass as bass
import concourse.tile as tile
from concourse import bacc, mybir

F32 = mybir.dt.float32
BF16 = mybir.dt.bfloat16
FP8 = mybir.dt.float8e4
U8 = mybir.dt.uint8
AX = mybir.AxisListType
AF = mybir.ActivationFunctionType
ALU = mybir.AluOpType
DR = mybir.MatmulPerfMode.DoubleRow

P = 128
EXP_BIAS = -2.0  # s' = exp(z - 2): keeps fp8 m0 in range; cancels in num/den


def make_consts(SEQ, n_cores):
    import ml_dtypes
    bf = lambda a: a.astype(ml_dtypes.bfloat16)
    f8 = lambda a: a.astype(ml_dtypes.float8_e4m3)
    T = SEQ // P
    NPAIR = T // 2
    ident = np.eye(P, dtype=np.float32)
    # PVt stationary: within-tile prefix stat[j, r] = 1 if j <= r  (triu)
    triu = np.triu(np.ones((P, P), np.float32))
    # selector+w [P, NPAIR, 2, 128]: col t (t<T) = [tile == t], col 64 =
    # (t_glob+1)/64.  (full 128 cols: dual-fp8 LdWeights rejects
    # partial-column tiles)
    selw = np.zeros((P, NPAIR, 2, 128), np.float32)
    for p_ in range(NPAIR):
        for s_ in range(2):
            selw[:, p_, s_, 2 * p_ + s_] = 1.0
            t_glob = 256 * p_ + 128 * s_ + np.arange(P)
            selw[:, p_, s_, 64] = (t_glob + 1.0) / 64.0
    # cross-tile strict prefix [t', t] = 1 if t' < t
    stril32T = np.triu(np.ones((T, T), np.float32), 1)
    return dict(
        c_identbf=bf(ident), c_ident=ident,
        c_triu8=f8(triu),
        c_selw=f8(selw.reshape(P, NPAIR * 2 * 128)),
        c_stril32T=bf(stril32T),
    )


def build(SEQ=4096, D=1024, n_cores=8):
    T = SEQ // P           # global 128-key tiles (32)
    TL = T // n_cores      # local tiles per core (4)
    B = P * TL             # rows per core (512)
    B2 = B // 2            # key half per core (256)
    DC = D // P            # feature chunks (8)
    NPAIR = T // 2         # global 256-key pairs (16)
    QC = B // P            # q chunks per core (4)
    assert B == 512 and DC == 8 and TL == 4
    scale = float(1.0 / np.sqrt(D) / 64.0)   # wq,wk each prescaled x8

    nc = bacc.Bacc("TRN2", target_bir_lowering=False, debug=False, num_devices=n_cores)

    x = nc.dram_tensor("x", [B, D], BF16, kind="ExternalInput")
    wq_d = nc.dram_tensor("wq", [P, DC * D], FP8, kind="ExternalInput")
    wk_d = nc.dram_tensor("wk", [P, DC * D], FP8, kind="ExternalInput")
    wv_d = nc.dram_tensor("wv", [P, DC * D], FP8, kind="ExternalInput")
    wvb_d = nc.dram_tensor("wvb", [P, DC * D], BF16, kind="ExternalInput")
    c_identbf = nc.dram_tensor("c_identbf", [P, P], BF16, kind="ExternalInput")
    c_ident = nc.dram_tensor("c_ident", [P, P], F32, kind="ExternalInput")
    c_triu8 = nc.dram_tensor("c_triu8", [P, P], FP8, kind="ExternalInput")
    c_selw = nc.dram_tensor("c_selw", [P, NPAIR * 2 * 128], FP8, kind="ExternalInput")
    c_stril32T = nc.dram_tensor("c_stril32T", [T, T], BF16, kind="ExternalInput")
    out = nc.dram_tensor("out", [B, D], F32, kind="ExternalOutput")

    KH = D * B2                  # one kT key-half, fp8 bytes
    DH = D // 2
    PVH = TL * P * DH            # PVt payload per d-half, fp8 bytes
    VSB = TL * D * 4             # VS payload, f32 bytes
    CC2AN = PVH + VSB
    CC2BN = PVH

    with tile.TileContext(nc) as tc, ExitStack() as top:
        dram1 = top.enter_context(tc.tile_pool(name="dram1", bufs=1, space="DRAM"))
        dram2a = top.enter_context(tc.tile_pool(name="dram2a", bufs=1, space="DRAM"))
        dram2b = top.enter_context(tc.tile_pool(name="dram2b", bufs=1, space="DRAM"))
        cc1_in = dram1.tile([2 * KH], FP8)
        cc1_out = dram1.tile([n_cores, 2 * KH], FP8, addr_space="Shared")
        cc2a_in = dram2a.tile([CC2AN], U8)
        cc2a_out = dram2a.tile([n_cores, CC2AN], U8, addr_space="Shared")
        cc2b_in = dram2b.tile([CC2BN], U8)
        cc2b_out = dram2b.tile([n_cores, CC2BN], U8, addr_space="Shared")
        consts = top.enter_context(tc.tile_pool(name="consts", bufs=1))
        identbf = consts.tile([P, P], BF16)
        nc.sync.dma_start(identbf[:], c_identbf.ap())
        triu8_sb = consts.tile([P, P], FP8)
        nc.scalar.dma_start(triu8_sb[:], c_triu8.ap())
        selw_sb = consts.tile([P, NPAIR * 2 * 128], FP8)
        nc.scalar.dma_start(selw_sb[:], c_selw.ap())
        stril32T_sb = consts.tile([T, T], BF16)
        nc.scalar.dma_start(stril32T_sb[:], c_stril32T.ap())
        ident_sb = consts.tile([P, P], F32)
        nc.scalar.dma_start(ident_sb[:], c_ident.ap())
        expb = consts.tile([P, 1], F32)
        nc.vector.memset(expb[:], EXP_BIAS)

        persist = top.enter_context(tc.tile_pool(name="persist", bufs=1))
        qT = persist.tile([P, DC * B], FP8)          # q.T row block
        m0 = persist.tile([P, NPAIR * 2 * B], FP8)   # exp scores, [key, pair, slot, q]
        pvgA = persist.tile([P, T * DH], FP8)        # gathered PV tiles, d-half 0
        pvgB = persist.tile([P, T * DH], FP8)        # gathered PV tiles, d-half 1
        rs_sb = persist.tile([T, B], BF16)           # per-tile rowsums of s
        cvs_sb = persist.tile([T, D], BF16)          # cross-tile prefix colsums
        vs_all = persist.tile([T, D], F32)
        vs_bf = persist.tile([T, D], BF16)
        recip = persist.tile([P, QC], F32)
        dennat = persist.tile([P, QC], F32)
        den_pad = persist.tile([P, B], F32)

        # ------------------- stage 1: local projections -------------------
        with ExitStack() as s1:
            xp = s1.enter_context(tc.tile_pool(name="xload", bufs=1))
            xt_sb = xp.tile([P, TL * D], BF16)       # x rows, [p, tile, d]
            nc.sync.dma_start(
                xt_sb.rearrange("p (t d) -> p t d", t=TL),
                x.ap().rearrange("(t p) d -> p t d", p=P))
            xT8 = xp.tile([P, DC * B], FP8)          # x.T, [d, dc, row]
            xTb = xp.tile([P, DC * B], BF16)         # x.T in bf16 (for xrs)
            wk_sb = xp.tile([P, DC * D], FP8)
            nc.sync.dma_start(wk_sb[:], wk_d.ap())
            wq_sb = xp.tile([P, DC * D], FP8)
            nc.scalar.dma_start(wq_sb[:], wq_d.ap())
            wv_sb = xp.tile([P, DC * D], FP8)
            nc.scalar.dma_start(wv_sb[:], wv_d.ap())
            wvb_sb = xp.tile([P, DC * D], BF16)
            nc.scalar.dma_start(wvb_sb[:], wvb_d.ap())
            kT_loc = xp.tile([P, DC * B], FP8)
            vpair = xp.tile([P, 2 * 2 * D], FP8)     # v tiles [row, pairidx, slot, d]
            xrs_bf = xp.tile([P, DC * TL], BF16)     # per-tile x row sums (.T)

            xt3 = xt_sb.rearrange("p (t d) -> p t d", t=TL)
            with ExitStack() as str_:
                trps = str_.enter_context(tc.tile_pool(name="trps", bufs=3, space="PSUM"))
                for dc in range(DC):
                    ps = trps.tile([P, B], F32, tag="tr")
                    for tcc in range(TL):
                        nc.tensor.matmul(ps[:, tcc * P:(tcc + 1) * P],
                                         xt3[:, tcc, dc * P:(dc + 1) * P], identbf[:],
                                         start=True, stop=True)
                    (nc.vector.tensor_copy if dc % 2 == 0 else nc.scalar.copy)(
                        xT8[:, dc * B:(dc + 1) * B], ps[:])
                    (nc.scalar.copy if dc % 2 == 0 else nc.vector.tensor_copy)(
                        xTb[:, dc * B:(dc + 1) * B], ps[:])

            pps = s1.enter_context(tc.tile_pool(name="pps", bufs=3, space="PSUM"))
            wk3 = wk_sb.rearrange("p (dc d) -> p dc d", dc=DC)
            wq3 = wq_sb.rearrange("p (dc d) -> p dc d", dc=DC)
            wv3 = wv_sb.rearrange("p (dc d) -> p dc d", dc=DC)
            wvb3 = wvb_sb.rearrange("p (dc d) -> p dc d", dc=DC)
            xT83 = xT8.rearrange("p (dc b) -> p dc b", dc=DC)

            # kT projection (gates the first collective)
            kT3 = kT_loc.rearrange("p (dc b) -> p dc b", dc=DC)
            for dco in range(DC):
                k_ps = pps.tile([P, B], F32, tag="pp", name="k_ps")
                for pp_ in range(DC // 2):
                    nc.tensor.matmul(
                        k_ps[:],
                        wk3[:, 2 * pp_:2 * pp_ + 2, dco * P:(dco + 1) * P],
                        xT83[:, 2 * pp_:2 * pp_ + 2, :],
                        start=(pp_ == 0), stop=(pp_ == DC // 2 - 1),
                        perf_mode=DR,
                    )
                nc.vector.tensor_copy(kT3[:, dco, :], k_ps[:])
            nc.sync.dma_start(
                cc1_in[:].rearrange("(dc p i) -> p dc i", dc=DC, p=P),
                kT3[:, :, :],
            )
            nc.gpsimd.collective_compute(
                "AllGather", ALU.bypass,
                replica_groups=[list(range(n_cores))],
                ins=[cc1_in.opt()], outs=[cc1_out.opt()],
            )

            # qT projection
            for dco in range(DC):
                q_ps = pps.tile([P, B], F32, tag="pp", name="q_ps")
                for pp_ in range(DC // 2):
                    nc.tensor.matmul(
                        q_ps[:],
                        wq3[:, 2 * pp_:2 * pp_ + 2, dco * P:(dco + 1) * P],
                        xT83[:, 2 * pp_:2 * pp_ + 2, :],
                        start=(pp_ == 0), stop=(pp_ == DC // 2 - 1),
                        perf_mode=DR,
                    )
                nc.vector.tensor_copy(qT[:, dco * B:(dco + 1) * B], q_ps[:])

            # per-tile x row sums (bf16, for the high-precision VS aggregates)
            xrs_f = xp.tile([P, DC * TL], F32)
            for dc in range(DC):
                nc.vector.reduce_sum(
                    xrs_f[:, dc * TL:(dc + 1) * TL],
                    xTb[:, dc * B:(dc + 1) * B].rearrange("p (t i) -> p t i", t=TL),
                    axis=AX.X,
                )
            nc.vector.tensor_copy(xrs_bf[:], xrs_f[:])

            # V projection, fp8 DR; v full scale (wv prescaled x8, cast /8)
            vp4 = vpair.rearrange("p (pr s d) -> p pr s d", pr=2, s=2)
            for tcc in range(TL):
                for g in range(2):
                    v_ps = pps.tile([P, D // 2], F32, tag="pp", name="v_ps")
                    for pp_ in range(DC // 2):
                        nc.tensor.matmul(
                            v_ps[:],
                            xT83[:, 2 * pp_:2 * pp_ + 2, tcc * P:(tcc + 1) * P],
                            wv3[:, 2 * pp_:2 * pp_ + 2, g * (D // 2):(g + 1) * (D // 2)],
                            start=(pp_ == 0), stop=(pp_ == DC // 2 - 1),
                            perf_mode=DR,
                        )
                    nc.vector.tensor_scalar(
                        vp4[:, tcc // 2, tcc % 2, g * (D // 2):(g + 1) * (D // 2)],
                        v_ps[:], 1.0 / 8.0, None, op0=ALU.mult)

            # VS aggregates: VS[t] = xrs[t] @ wv_bf  ([TL, D], bf16 path)
            vs_loc = xp.tile([TL, D], F32)
            for g in range(2):
                dsl = slice(g * (D // 2), (g + 1) * (D // 2))
                vs_ps = pps.tile([TL, D // 2], F32, tag="pp", name=f"vs_ps{g}")
                for dc in range(DC):
                    nc.tensor.matmul(
                        vs_ps[:], xrs_bf[:, dc * TL:(dc + 1) * TL], wvb3[:, dc, dsl],
                        start=(dc == 0), stop=(dc == DC - 1),
                    )
                nc.vector.tensor_copy(vs_loc[:, dsl], vs_ps[:])

            # PVt: within-tile prefix sums of v (fp8 in/out, f32 psum),
            # d-half at a time; each half ships in its own collective
            pv_locA = xp.tile([P, TL * DH], FP8)
            pv_locB = xp.tile([P, TL * DH], FP8)
            with ExitStack() as spv:
                pvtp = spv.enter_context(tc.tile_pool(name="pvtp", bufs=4, space="PSUM"))
                for g, (pvl, cc_in) in enumerate(
                        [(pv_locA, cc2a_in), (pv_locB, cc2b_in)]):
                    pvl3 = pvl.rearrange("p (t d) -> p t d", t=TL)
                    dsl = slice(g * DH, (g + 1) * DH)
                    for tcc in range(TL):
                        pv_ps = pvtp.tile([P, DH], F32, tag="pv")
                        nc.tensor.matmul(
                            pv_ps[:], triu8_sb[:], vp4[:, tcc // 2, tcc % 2, dsl],
                            start=True, stop=True,
                        )
                        nc.vector.tensor_copy(pvl3[:, tcc, :], pv_ps[:])
                    nc.sync.dma_start(
                        cc_in[0:PVH].rearrange("(t p d) -> p t d", t=TL, p=P),
                        pvl3.bitcast(U8),
                    )
                    if g == 0:
                        nc.sync.dma_start(
                            cc2a_in[PVH:].rearrange("(t d) -> t d", t=TL),
                            vs_loc[:].bitcast(U8),
                        )
                    nc.gpsimd.collective_compute(
                        "AllGather", ALU.bypass,
                        replica_groups=[list(range(n_cores))],
                        ins=[cc_in.opt()],
                        outs=[[cc2a_out, cc2b_out][g].opt()],
                    )

        # ------------------- phase A: scores / exp / selector -------------------
        m04 = m0.rearrange("p (pr s b) -> p pr s b", pr=NPAIR, s=2)
        qT3 = qT.rearrange("p (dc b) -> p dc b", dc=DC)
        selw4 = selw_sb.rearrange("p (pr s c) -> p pr s c", pr=NPAIR, s=2)
        with ExitStack() as pa:
            ktp = pa.enter_context(tc.tile_pool(name="kt", bufs=4))
            ztp = pa.enter_context(tc.tile_pool(name="zt", bufs=3, space="PSUM"))
            rdp = pa.enter_context(tc.tile_pool(name="rd", bufs=1, space="PSUM"))
            rd_ps = rdp.tile([P, B], F32)

            for rc in range(n_cores):
                ktc = ktp.tile([P, DC * B], FP8, tag="kt")
                nc.sync.dma_start(
                    ktc.rearrange("p (dc i) -> p dc i", dc=DC),
                    cc1_out[rc, :].rearrange("(dc p i) -> p dc i", dc=DC, p=P),
                )
                ktc3 = ktc.rearrange("p (dc i) -> p dc i", dc=DC)
                for s_ in range(2 * 2):
                    pair = rc * 2 + s_ // 2
                    zt = ztp.tile([P, B], F32, tag="zt")
                    for pp in range(DC // 2):
                        nc.tensor.matmul(
                            zt[:],
                            ktc3[:, 2 * pp:2 * pp + 2, s_ * P:(s_ + 1) * P],
                            qT3[:, 2 * pp:2 * pp + 2, :],
                            start=(pp == 0), stop=(pp == DC // 2 - 1),
                            perf_mode=DR,
                        )
                    nc.scalar.activation(
                        m04[:, pair, s_ % 2, :], zt[:], AF.Exp,
                        bias=expb[:], scale=scale)
                    if s_ % 2 == 1:
                        nc.tensor.matmul(
                            rd_ps[:], selw4[:, pair, :, :], m04[:, pair, :, :],
                            start=(rc == 0 and s_ == 1),
                            stop=(rc == n_cores - 1 and s_ == 3),
                            perf_mode=DR,
                        )

            # rs (per-tile rowsums) + den out of the selector psum
            nc.vector.tensor_copy(rs_sb[:], rd_ps[0:T, :])
            nc.vector.memset(den_pad[:], 0.0)
            nc.vector.tensor_copy(den_pad[64:65, :], rd_ps[64:65, :])
            for qc in range(QC):
                dps = ztp.tile([P, P], F32, tag="zt")
                nc.tensor.transpose(dps[:], den_pad[:, qc * P:(qc + 1) * P], ident_sb[:])
                nc.vector.tensor_copy(dennat[:, qc:qc + 1], dps[:, 64:65])
            nc.vector.reciprocal(recip[:], dennat[:])

            # CVS = stril32 @ VS_all (bf16), needs gather 2a
            for rc in range(n_cores):
                nc.sync.dma_start(
                    vs_all[rc * TL:(rc + 1) * TL, :].bitcast(U8),
                    cc2a_out[rc, PVH:].rearrange("(t d) -> t d", t=TL),
                )
            nc.vector.tensor_copy(vs_bf[:], vs_all[:])
            for g in range(2):
                cvs_ps = rdp.tile([T, D // 2], F32, tag="rd2", name=f"cvs_ps{g}")
                nc.tensor.matmul(cvs_ps[:], stril32T_sb[:],
                                 vs_bf[:, g * 512:(g + 1) * 512],
                                 start=True, stop=True)
                nc.vector.tensor_copy(cvs_sb[:, g * 512:(g + 1) * 512], cvs_ps[:])

        # gathered PV tiles -> SBUF (per remote core; two queues)
        for rc in range(n_cores):
            nc.sync.dma_start(
                pvgA[:, rc * TL * DH:(rc + 1) * TL * DH]
                .rearrange("p (t d) -> p t d", t=TL).bitcast(U8),
                cc2a_out[rc, 0:PVH].rearrange("(t p d) -> p t d", t=TL, p=P),
            )
            nc.scalar.dma_start(
                pvgB[:, rc * TL * DH:(rc + 1) * TL * DH]
                .rearrange("p (t d) -> p t d", t=TL).bitcast(U8),
                cc2b_out[rc, 0:PVH].rearrange("(t p d) -> p t d", t=TL, p=P),
            )

        # ------------------- phase B: num accumulation -------------------
        pvgA4 = pvgA.rearrange("p (pr s d) -> p pr s d", pr=NPAIR, s=2)
        pvgB4 = pvgB.rearrange("p (pr s d) -> p pr s d", pr=NPAIR, s=2)
        with ExitStack() as pb:
            nump = pb.enter_context(tc.tile_pool(name="nump", bufs=8, space="PSUM"))
            osb = pb.enter_context(tc.tile_pool(name="osb", bufs=4))
            for g in range(2):
                dsl = slice(g * 512, (g + 1) * 512)
                nums = [nump.tile([P, 512], F32, tag="num", name=f"num{g}_{qc}")
                        for qc in range(QC)]
                pvg4 = [pvgA4, pvgB4][g]
                for pair in range(NPAIR):
                    for qc in range(QC):
                        nc.tensor.matmul(
                            nums[qc][:],
                            m04[:, pair, :, qc * P:(qc + 1) * P],
                            pvg4[:, pair, :, :],
                            start=(pair == 0), stop=False,
                            perf_mode=DR,
                        )
                for qc in range(QC):
                    nc.tensor.matmul(
                        nums[qc][:],
                        rs_sb[:, qc * P:(qc + 1) * P], cvs_sb[:, dsl],
                        start=False, stop=True,
                    )
                for qc in range(QC):
                    ot = osb.tile([P, 512], F32, tag="ot")
                    nc.vector.tensor_scalar(
                        ot[:], nums[qc][:], recip[:, qc:qc + 1], 1.0 / 64.0,
                        op0=ALU.mult, op1=ALU.mult)
                    nc.scalar.dma_start(out.ap()[qc * P:(qc + 1) * P, dsl], ot[:])

    nc.compile()
    return nc


def make_in_maps(x_full, wq, wk, wv, n_cores=8):
    import ml_dtypes
    bf = lambda a: np.ascontiguousarray(a).astype(ml_dtypes.bfloat16)
    f8 = lambda a: np.ascontiguousarray(a).astype(ml_dtypes.float8_e4m3)
    SEQ, D = x_full.shape
    DC = D // P
    B = SEQ // n_cores
    consts = make_consts(SEQ, n_cores)
    # weight images: [p, dc*D + j] = w[dc*128 + p, j]; fp8 ones prescaled x8
    img8 = lambda w: f8((w * 8.0).reshape(DC, P, D).transpose(1, 0, 2).reshape(P, DC * D))
    imgb = lambda w: bf(w.reshape(DC, P, D).transpose(1, 0, 2).reshape(P, DC * D))
    in_maps = []
    for c in range(n_cores):
        m = {"x": bf(x_full[c * B:(c + 1) * B]),
             "wq": img8(wq), "wk": img8(wk), "wv": img8(wv), "wvb": imgb(wv)}
        m.update(consts)
        in_maps.append(m)
    return in_maps


def algo_ref(x, wq, wk, wv):
    """Numpy float64 reference of the restructured math (for validation)."""
    x = x.astype(np.float64)
    q = x @ wq.astype(np.float64)
    k = x @ wk.astype(np.float64)
    v = x @ wv.astype(np.float64)
    z = q @ k.T / np.sqrt(k.shape[1])
    s = np.exp(z)
    pv = np.cumsum(v, axis=0)
    num = s @ pv
    den = s @ (np.arange(1, x.shape[0] + 1, dtype=np.float64))
    return (num / den[:, None]).astype(np.float32)


# ----------------------------------------------------------------------------
# Harness entry point: full (unsharded) inputs -> full output.
# ----------------------------------------------------------------------------
SEQ, D_IN, N_CORES = 4096, 1024, 8
_built = {}


def _get_nc(SEQ_=SEQ, D_=D_IN, n_cores=N_CORES):
    key = (SEQ_, D_, n_cores)
    if key not in _built:
        _built[key] = build(SEQ=SEQ_, D=D_, n_cores=n_cores)
    return _built[key]


def run(x, wq, wk, wv, trace=False, **spmd_kwargs):
    from concourse.bass_utils import run_bass_kernel_spmd

    x = np.ascontiguousarray(np.asarray(x, dtype=np.float32))
    wq = np.ascontiguousarray(np.asarray(wq, dtype=np.float32))
    wk = np.ascontiguousarray(np.asarray(wk, dtype=np.float32))
    wv = np.ascontiguousarray(np.asarray(wv, dtype=np.float32))
    n_cores = N_CORES
    nc = _get_nc(x.shape[0], x.shape[1], n_cores)
    in_maps = make_in_maps(x, wq, wk, wv, n_cores=n_cores)
    res = run_bass_kernel_spmd(nc, in_maps, list(range(n_cores)),
                               trace=trace, **spmd_kwargs)
    out = np.concatenate([res.results[c]["out"] for c in range(n_cores)], axis=0)
    return out, res


def kernel(x, wq, wk, wv):
    out, _ = run(x, wq, wk, wv, trace=False)
    return out


# revision 24
# speedup vs baseline: 1.0452x; 1.0452x over previous
"""Bass/Tile TRN2 kernel for nn_CausalAttention (softmax + tril-matmul renorm).

V3 restructure around the identity  masked @ v == s @ cumsum(v):
    out[i] = (sum_t s[i,t] * PV[t]) / (sum_t (t+1) * s[i,t]),   s = exp(q k^T / sqrt(D))
with PV[t] = prefix-sum of v rows, split (for fp8 precision) as
    PV[t] = PVt_within[t] + CVS[tile(t)]
where PVt_within is the within-128-tile prefix (small magnitudes, fp8 ok) and
CVS carries all cross-tile aggregates in bf16 (tile colsums VS computed in
high precision from x row-sums: VS = xrs @ wv_bf).

Per core (512 q rows):
  local:  xT (transposes, bf16+fp8), kT/qT (fp8 DR), v (fp8 DR, full scale),
          PVt = within-tile prefix of v (triu matmuls), xrs row sums,
          VS = xrsT-chunks @ wv_bf (bf16)
  comm:   AllGather kT in two key-halves (scores start on half 1),
          AllGather [PVt fp8 | VS f32]
  A:      zT tiles [key,q] (fp8 DR), exp -> m0 fp8, per-pair selector matmul
          -> rs (per-tile rowsums of s, rows 0..31) + den (row 64)
  B:      num[q,d] = sum_pairs m0_pair^T mm PV_pair (fp8 DR)
          + rank-32 close: rs^T mm CVS (bf16), CVS = stril32 @ VS_all
          out = num * recip(den) / 64
"""
import numpy as np
from contextlib import ExitStack

import concourse.bass as bass
import concourse.tile as tile
from concourse import bacc, mybir

F32 = mybir.dt.float32
BF16 = mybir.dt.bfloat16
FP8 = mybir.dt.float8e4
U8 = mybir.dt.uint8
AX = mybir.AxisListType
AF = mybir.ActivationFunctionType
ALU = mybir.AluOpType
DR = mybir.MatmulPerfMode.DoubleRow

P = 128
EXP_BIAS = -2.0  # s' = exp(z - 2): keeps fp8 m0 in range; cancels in num/den


def make_consts(SEQ, n_cores):
    import ml_dtypes
    bf = lambda a: a.astype(ml_dtypes.bfloat16)
    f8 = lambda a: a.astype(ml_dtypes.float8_e4m3)
    T = SEQ // P
    NPAIR = T // 2
    ident = np.eye(P, dtype=np.float32)
    # PVt stationary: within-tile prefix stat[j, r] = 1 if j <= r  (triu)
    triu = np.triu(np.ones((P, P), np.float32))
    # selector+w [P, NPAIR, 2, 128]: col t (t<T) = [tile == t], col 64 =
    # (t_glob+1)/64.  (full 128 cols: dual-fp8 LdWeights rejects
    # partial-column tiles)
    selw = np.zeros((P, NPAIR, 2, 128), np.float32)
    for p_ in range(NPAIR):
        for s_ in range(2):
            selw[:, p_, s_, 2 * p_ + s_] = 1.0
            t_glob = 256 * p_ + 128 * s_ + np.arange(P)
            selw[:, p_, s_, 64] = (t_glob + 1.0) / 64.0
    # cross-tile strict prefix [t', t] = 1 if t' < t
    stril32T = np.triu(np.ones((T, T), np.float32), 1)
    return dict(
        c_identbf=bf(ident), c_ident=ident,
        c_triu8=f8(triu),
        c_selw=f8(selw.reshape(P, NPAIR * 2 * 128)),
        c_stril32T=bf(stril32T),
    )


def build(SEQ=4096, D=1024, n_cores=8):
    T = SEQ // P           # global 128-key tiles (32)
    TL = T // n_cores      # local tiles per core (4)
    B = P * TL             # rows per core (512)
    B2 = B // 2            # key half per core (256)
    DC = D // P            # feature chunks (8)
    NPAIR = T // 2         # global 256-key pairs (16)
    QC = B // P            # q chunks per core (4)
    assert B == 512 and DC == 8 and TL == 4
    scale = float(1.0 / np.sqrt(D) / 64.0)   # wq,wk each prescaled x8

    nc = bacc.Bacc("TRN2", target_bir_lowering=False, debug=False, num_devices=n_cores)

    x = nc.dram_tensor("x", [B, D], BF16, kind="ExternalInput")
    wq_d = nc.dram_tensor("wq", [P, DC * D], FP8, kind="ExternalInput")
    wk_d = nc.dram_tensor("wk", [P, DC * D], FP8, kind="ExternalInput")
    wv_d = nc.dram_tensor("wv", [P, DC * D], FP8, kind="ExternalInput")
    wvb_d = nc.dram_tensor("wvb", [P, DC * D], BF16, kind="ExternalInput")
    c_identbf = nc.dram_tensor("c_identbf", [P, P], BF16, kind="ExternalInput")
    c_ident = nc.dram_tensor("c_ident", [P, P], F32, kind="ExternalInput")
    c_triu8 = nc.dram_tensor("c_triu8", [P, P], FP8, kind="ExternalInput")
    c_selw = nc.dram_tensor("c_selw", [P, NPAIR * 2 * 128], FP8, kind="ExternalInput")
    c_stril32T = nc.dram_tensor("c_stril32T", [T, T], BF16, kind="ExternalInput")
    out = nc.dram_tensor("out", [B, D], F32, kind="ExternalOutput")

    KH = D * B2                  # one kT key-half, fp8 bytes
    DH = D // 2
    PVH = TL * P * DH            # PVt payload per d-half, fp8 bytes
    VSB = TL * D * 4             # VS payload, f32 bytes
    CC2AN = PVH + VSB
    CC2BN = PVH

    with tile.TileContext(nc) as tc, ExitStack() as top:
        dram1 = top.enter_context(tc.tile_pool(name="dram1", bufs=1, space="DRAM"))
        dram2a = top.enter_context(tc.tile_pool(name="dram2a", bufs=1, space="DRAM"))
        dram2b = top.enter_context(tc.tile_pool(name="dram2b", bufs=1, space="DRAM"))
        cc1_in = dram1.tile([2 * KH], FP8)
        cc1_out = dram1.tile([n_cores, 2 * KH], FP8, addr_space="Shared")
        cc2a_in = dram2a.tile([CC2AN], U8)
        cc2a_out = dram2a.tile([n_cores, CC2AN], U8, addr_space="Shared")
        cc2b_in = dram2b.tile([CC2BN], U8)
        cc2b_out = dram2b.tile([n_cores, CC2BN], U8, addr_space="Shared")
        consts = top.enter_context(tc.tile_pool(name="consts", bufs=1))
        identbf = consts.tile([P, P], BF16)
        nc.sync.dma_start(identbf[:], c_identbf.ap())
        triu8_sb = consts.tile([P, P], FP8)
        nc.scalar.dma_start(triu8_sb[:], c_triu8.ap())
        selw_sb = consts.tile([P, NPAIR * 2 * 128], FP8)
        nc.scalar.dma_start(selw_sb[:], c_selw.ap())
        stril32T_sb = consts.tile([T, T], BF16)
        nc.scalar.dma_start(stril32T_sb[:], c_stril32T.ap())
        ident_sb = consts.tile([P, P], F32)
        nc.scalar.dma_start(ident_sb[:], c_ident.ap())
        expb = consts.tile([P, 1], F32)
        nc.vector.memset(expb[:], EXP_BIAS)

        persist = top.enter_context(tc.tile_pool(name="persist", bufs=1))
        qT = persist.tile([P, DC * B], FP8)          # q.T row block
        m0 = persist.tile([P, NPAIR * 2 * B], FP8)   # exp scores, [key, pair, slot, q]
        pvgA = persist.tile([P, T * DH], FP8)        # gathered PV tiles, d-half 0
        pvgB = persist.tile([P, T * DH], FP8)        # gathered PV tiles, d-half 1
        rs_sb = persist.tile([T, B], BF16)           # per-tile rowsums of s
        cvs_sb = persist.tile([T, D], BF16)          # cross-tile prefix colsums
        vs_all = persist.tile([T, D], F32)
        vs_bf = persist.tile([T, D], BF16)
        recip = persist.tile([P, QC], F32)
        dennat = persist.tile([P, QC], F32)
        den_pad = persist.tile([P, B], F32)

        # ------------------- stage 1: local projections -------------------
        with ExitStack() as s1:
            xp = s1.enter_context(tc.tile_pool(name="xload", bufs=1))
            xt_sb = xp.tile([P, TL * D], BF16)       # x rows, [p, tile, d]
            for xh in range(2):
                nc.sync.dma_start(
                    xt_sb.rearrange("p (t d) -> p t d", t=TL)[:, xh * 2:(xh + 1) * 2, :],
                    x.ap().rearrange("(t p) d -> p t d", p=P)[:, xh * 2:(xh + 1) * 2, :])
            xT8 = xp.tile([P, DC * B], FP8)          # x.T, [d, dc, row]
            xTb = xp.tile([P, DC * B], BF16)         # x.T in bf16 (for xrs)
            wk_sb = xp.tile([P, DC * D], FP8)
            nc.sync.dma_start(wk_sb[:], wk_d.ap())
            wq_sb = xp.tile([P, DC * D], FP8)
            nc.scalar.dma_start(wq_sb[:], wq_d.ap())
            wv_sb = xp.tile([P, DC * D], FP8)
            nc.scalar.dma_start(wv_sb[:], wv_d.ap())
            wvb_sb = xp.tile([P, DC * D], BF16)
            nc.scalar.dma_start(wvb_sb[:], wvb_d.ap())
            kT_loc = xp.tile([P, DC * B], FP8)
            vpair = xp.tile([P, 2 * 2 * D], FP8)     # v tiles [row, pairidx, slot, d]
            xrs_bf = xp.tile([P, DC * TL], BF16)     # per-tile x row sums (.T)

            xt3 = xt_sb.rearrange("p (t d) -> p t d", t=TL)
            with ExitStack() as str_:
                trps = str_.enter_context(tc.tile_pool(name="trps", bufs=3, space="PSUM"))
                for dc in range(DC):
                    ps = trps.tile([P, B], F32, tag="tr")
                    for tcc in range(TL):
                        nc.tensor.matmul(ps[:, tcc * P:(tcc + 1) * P],
                                         xt3[:, tcc, dc * P:(dc + 1) * P], identbf[:],
                                         start=True, stop=True)
                    (nc.vector.tensor_copy if dc % 2 == 0 else nc.scalar.copy)(
                        xT8[:, dc * B:(dc + 1) * B], ps[:])
                    (nc.scalar.copy if dc % 2 == 0 else nc.vector.tensor_copy)(
                        xTb[:, dc * B:(dc + 1) * B], ps[:])

            pps = s1.enter_context(tc.tile_pool(name="pps", bufs=3, space="PSUM"))
            wk3 = wk_sb.rearrange("p (dc d) -> p dc d", dc=DC)
            wq3 = wq_sb.rearrange("p (dc d) -> p dc d", dc=DC)
            wv3 = wv_sb.rearrange("p (dc d) -> p dc d", dc=DC)
            wvb3 = wvb_sb.rearrange("p (dc d) -> p dc d", dc=DC)
            xT83 = xT8.rearrange("p (dc b) -> p dc b", dc=DC)

            # kT projection (gates the first collective)
            kT3 = kT_loc.rearrange("p (dc b) -> p dc b", dc=DC)
            for dco in range(DC):
                k_ps = pps.tile([P, B], F32, tag="pp", name="k_ps")
                for pp_ in range(DC // 2):
                    nc.tensor.matmul(
                        k_ps[:],
                        wk3[:, 2 * pp_:2 * pp_ + 2, dco * P:(dco + 1) * P],
                        xT83[:, 2 * pp_:2 * pp_ + 2, :],
                        start=(pp_ == 0), stop=(pp_ == DC // 2 - 1),
                        perf_mode=DR,
                    )
                nc.vector.tensor_copy(kT3[:, dco, :], k_ps[:])
            nc.sync.dma_start(
                cc1_in[:].rearrange("(dc p i) -> p dc i", dc=DC, p=P),
                kT3[:, :, :],
            )
            nc.gpsimd.collective_compute(
                "AllGather", ALU.bypass,
                replica_groups=[list(range(n_cores))],
                ins=[cc1_in.opt()], outs=[cc1_out.opt()],
            )

            # qT projection
            for dco in range(DC):
                q_ps = pps.tile([P, B], F32, tag="pp", name="q_ps")
                for pp_ in range(DC // 2):
                    nc.tensor.matmul(
                        q_ps[:],
                        wq3[:, 2 * pp_:2 * pp_ + 2, dco * P:(dco + 1) * P],
                        xT83[:, 2 * pp_:2 * pp_ + 2, :],
                        start=(pp_ == 0), stop=(pp_ == DC // 2 - 1),
                        perf_mode=DR,
                    )
                nc.vector.tensor_copy(qT[:, dco * B:(dco + 1) * B], q_ps[:])

            # per-tile x row sums (bf16, for the high-precision VS aggregates)
            xrs_f = xp.tile([P, DC * TL], F32)
            for dc in range(DC):
                nc.vector.reduce_sum(
                    xrs_f[:, dc * TL:(dc + 1) * TL],
                    xTb[:, dc * B:(dc + 1) * B].rearrange("p (t i) -> p t i", t=TL),
                    axis=AX.X,
                )
            nc.vector.tensor_copy(xrs_bf[:], xrs_f[:])

            # V projection, fp8 DR; v full scale (wv prescaled x8, cast /8)
            vp4 = vpair.rearrange("p (pr s d) -> p pr s d", pr=2, s=2)
            for tcc in range(TL):
                for g in range(2):
                    v_ps = pps.tile([P, D // 2], F32, tag="pp", name="v_ps")
                    for pp_ in range(DC // 2):
                        nc.tensor.matmul(
                            v_ps[:],
                            xT83[:, 2 * pp_:2 * pp_ + 2, tcc * P:(tcc + 1) * P],
                            wv3[:, 2 * pp_:2 * pp_ + 2, g * (D // 2):(g + 1) * (D // 2)],
                            start=(pp_ == 0), stop=(pp_ == DC // 2 - 1),
                            perf_mode=DR,
                        )
                    nc.vector.tensor_scalar(
                        vp4[:, tcc // 2, tcc % 2, g * (D // 2):(g + 1) * (D // 2)],
                        v_ps[:], 1.0 / 8.0, None, op0=ALU.mult)

            # VS aggregates: VS[t] = xrs[t] @ wv_bf  ([TL, D], bf16 path)
            vs_loc = xp.tile([TL, D], F32)
            for g in range(2):
                dsl = slice(g * (D // 2), (g + 1) * (D // 2))
                vs_ps = pps.tile([TL, D // 2], F32, tag="pp", name=f"vs_ps{g}")
                for dc in range(DC):
                    nc.tensor.matmul(
                        vs_ps[:], xrs_bf[:, dc * TL:(dc + 1) * TL], wvb3[:, dc, dsl],
                        start=(dc == 0), stop=(dc == DC - 1),
                    )
                nc.vector.tensor_copy(vs_loc[:, dsl], vs_ps[:])

            # PVt: within-tile prefix sums of v (fp8 in/out, f32 psum),
            # d-half at a time; each half ships in its own collective
            pv_locA = xp.tile([P, TL * DH], FP8)
            pv_locB = xp.tile([P, TL * DH], FP8)
            with ExitStack() as spv:
                pvtp = spv.enter_context(tc.tile_pool(name="pvtp", bufs=4, space="PSUM"))
                for g, (pvl, cc_in) in enumerate(
                        [(pv_locA, cc2a_in), (pv_locB, cc2b_in)]):
                    pvl3 = pvl.rearrange("p (t d) -> p t d", t=TL)
                    dsl = slice(g * DH, (g + 1) * DH)
                    for tcc in range(TL):
                        pv_ps = pvtp.tile([P, DH], F32, tag="pv")
                        nc.tensor.matmul(
                            pv_ps[:], triu8_sb[:], vp4[:, tcc // 2, tcc % 2, dsl],
                            start=True, stop=True,
                        )
                        nc.vector.tensor_copy(pvl3[:, tcc, :], pv_ps[:])
                    nc.sync.dma_start(
                        cc_in[0:PVH].rearrange("(t p d) -> p t d", t=TL, p=P),
                        pvl3.bitcast(U8),
                    )
                    if g == 0:
                        nc.sync.dma_start(
                            cc2a_in[PVH:].rearrange("(t d) -> t d", t=TL),
                            vs_loc[:].bitcast(U8),
                        )
                    nc.gpsimd.collective_compute(
                        "AllGather", ALU.bypass,
                        replica_groups=[list(range(n_cores))],
                        ins=[cc_in.opt()],
                        outs=[[cc2a_out, cc2b_out][g].opt()],
                    )

        # gathered VS + PV tiles -> SBUF (issued up front; wait on gathers 2a/2b)
        for rc in range(n_cores):
            nc.scalar.dma_start(
                vs_all[rc * TL:(rc + 1) * TL, :].bitcast(U8),
                cc2a_out[rc, PVH:].rearrange("(t d) -> t d", t=TL),
            )
        for rc in range(n_cores):
            nc.scalar.dma_start(
                pvgA[:, rc * TL * DH:(rc + 1) * TL * DH]
                .rearrange("p (t d) -> p t d", t=TL).bitcast(U8),
                cc2a_out[rc, 0:PVH].rearrange("(t p d) -> p t d", t=TL, p=P),
            )
            nc.scalar.dma_start(
                pvgB[:, rc * TL * DH:(rc + 1) * TL * DH]
                .rearrange("p (t d) -> p t d", t=TL).bitcast(U8),
                cc2b_out[rc, 0:PVH].rearrange("(t p d) -> p t d", t=TL, p=P),
            )

        # ------------- phase A (scores/selector) + fused phase B -------------
        m04 = m0.rearrange("p (pr s b) -> p pr s b", pr=NPAIR, s=2)
        qT3 = qT.rearrange("p (dc b) -> p dc b", dc=DC)
        selw4 = selw_sb.rearrange("p (pr s c) -> p pr s c", pr=NPAIR, s=2)
        pvgA4 = pvgA.rearrange("p (pr s d) -> p pr s d", pr=NPAIR, s=2)
        pvgB4 = pvgB.rearrange("p (pr s d) -> p pr s d", pr=NPAIR, s=2)
        with ExitStack() as pa:
            ktp = pa.enter_context(tc.tile_pool(name="kt", bufs=4))
            ztp = pa.enter_context(tc.tile_pool(name="zt", bufs=3, space="PSUM"))
            rdp = pa.enter_context(tc.tile_pool(name="rd", bufs=1, space="PSUM"))
            nump = pa.enter_context(tc.tile_pool(name="nump", bufs=4, space="PSUM"))
            osb = pa.enter_context(tc.tile_pool(name="osb", bufs=4))
            rd_ps = rdp.tile([P, B], F32, tag="rd")

            for rc in range(n_cores):
                ktc = ktp.tile([P, DC * B], FP8, tag="kt")
                nc.sync.dma_start(
                    ktc.rearrange("p (dc i) -> p dc i", dc=DC),
                    cc1_out[rc, :].rearrange("(dc p i) -> p dc i", dc=DC, p=P),
                )
                ktc3 = ktc.rearrange("p (dc i) -> p dc i", dc=DC)
                for s_ in range(2 * 2):
                    pair = rc * 2 + s_ // 2
                    zt = ztp.tile([P, B], F32, tag="zt")
                    for pp in range(DC // 2):
                        nc.tensor.matmul(
                            zt[:],
                            ktc3[:, 2 * pp:2 * pp + 2, s_ * P:(s_ + 1) * P],
                            qT3[:, 2 * pp:2 * pp + 2, :],
                            start=(pp == 0), stop=(pp == DC // 2 - 1),
                            perf_mode=DR,
                        )
                    nc.scalar.activation(
                        m04[:, pair, s_ % 2, :], zt[:], AF.Exp,
                        bias=expb[:], scale=scale)
                    if s_ % 2 == 1:
                        nc.tensor.matmul(
                            rd_ps[:], selw4[:, pair, :, :], m04[:, pair, :, :],
                            start=(rc == 0 and s_ == 1),
                            stop=(rc == n_cores - 1 and s_ == 3),
                            perf_mode=DR,
                        )

            # rs (per-tile rowsums) + den out of the selector psum
            nc.vector.tensor_copy(rs_sb[:], rd_ps[0:T, :])
            nc.vector.memset(den_pad[:], 0.0)
            nc.vector.tensor_copy(den_pad[64:65, :], rd_ps[64:65, :])
            nc.vector.tensor_copy(vs_bf[:], vs_all[:])

            # phase B: g0 pair matmuls run while den/recip/CVS settle
            for g in range(2):
                dsl = slice(g * 512, (g + 1) * 512)
                nums = [nump.tile([P, 512], F32, tag="num", name=f"num{g}_{qc}")
                        for qc in range(QC)]
                pvg4 = [pvgA4, pvgB4][g]
                for pair in range(NPAIR):
                    for qc in range(QC):
                        nc.tensor.matmul(
                            nums[qc][:],
                            m04[:, pair, :, qc * P:(qc + 1) * P],
                            pvg4[:, pair, :, :],
                            start=(pair == 0), stop=False,
                            perf_mode=DR,
                        )
                if g == 0:
                    # den -> recip and CVS, in the shadow of g0's matmuls
                    for qc in range(QC):
                        dps = ztp.tile([P, P], F32, tag="zt")
                        nc.tensor.transpose(dps[:], den_pad[:, qc * P:(qc + 1) * P],
                                            ident_sb[:])
                        nc.vector.tensor_copy(dennat[:, qc:qc + 1], dps[:, 64:65])
                    nc.vector.reciprocal(recip[:], dennat[:])
                    for gc in range(2):
                        cvs_ps = rdp.tile([T, D // 2], F32, tag="rd",
                                          name=f"cvs_ps{gc}")
                        nc.tensor.matmul(cvs_ps[:], stril32T_sb[:],
                                         vs_bf[:, gc * 512:(gc + 1) * 512],
                                         start=True, stop=True)
                        nc.vector.tensor_copy(cvs_sb[:, gc * 512:(gc + 1) * 512],
                                              cvs_ps[:])
                for qc in range(QC):
                    nc.tensor.matmul(
                        nums[qc][:],
                        rs_sb[:, qc * P:(qc + 1) * P], cvs_sb[:, dsl],
                        start=False, stop=True,
                    )
                for qc in range(QC):
                    ot = osb.tile([P, 512], F32, tag="ot")
                    nc.vector.tensor_scalar(
                        ot[:], nums[qc][:], recip[:, qc:qc + 1], 1.0 / 64.0,
                        op0=ALU.mult, op1=ALU.mult)
                    nc.scalar.dma_start(out.ap()[qc * P:(qc + 1) * P, dsl], ot[:])

    nc.compile()
    return nc


def make_in_maps(x_full, wq, wk, wv, n_cores=8):
    import ml_dtypes
    bf = lambda a: np.ascontiguousarray(a).astype(ml_dtypes.bfloat16)
    f8 = lambda a: np.ascontiguousarray(a).astype(ml_dtypes.float8_e4m3)
    SEQ, D = x_full.shape
    DC = D // P
    B = SEQ // n_cores
    consts = make_consts(SEQ, n_cores)
    # weight images: [p, dc*D + j] = w[dc*128 + p, j]; fp8 ones prescaled x8
    img8 = lambda w: f8((w * 8.0).reshape(DC, P, D).transpose(1, 0, 2).reshape(P, DC * D))
    imgb = lambda w: bf(w.reshape(DC, P, D).transpose(1, 0, 2).reshape(P, DC * D))
    in_maps = []
    for c in range(n_cores):
        m = {"x": bf(x_full[c * B:(c + 1) * B]),
             "wq": img8(wq), "wk": img8(wk), "wv": img8(wv), "wvb": imgb(wv)}
        m.update(consts)
        in_maps.append(m)
    return in_maps


def algo_ref(x, wq, wk, wv):
    """Numpy float64 reference of the restructured math (for validation)."""
    x = x.astype(np.float64)
    q = x @ wq.astype(np.float64)
    k = x @ wk.astype(np.float64)
    v = x @ wv.astype(np.float64)
    z = q @ k.T / np.sqrt(k.shape[1])
    s = np.exp(z)
    pv = np.cumsum(v, axis=0)
    num = s @ pv
    den = s @ (np.arange(1, x.shape[0] + 1, dtype=np.float64))
    return (num / den[:, None]).astype(np.float32)


# ----------------------------------------------------------------------------
# Harness entry point: full (unsharded) inputs -> full output.
# ----------------------------------------------------------------------------
SEQ, D_IN, N_CORES = 4096, 1024, 8
_built = {}


def _get_nc(SEQ_=SEQ, D_=D_IN, n_cores=N_CORES):
    key = (SEQ_, D_, n_cores)
    if key not in _built:
        _built[key] = build(SEQ=SEQ_, D=D_, n_cores=n_cores)
    return _built[key]


def run(x, wq, wk, wv, trace=False, **spmd_kwargs):
    from concourse.bass_utils import run_bass_kernel_spmd

    x = np.ascontiguousarray(np.asarray(x, dtype=np.float32))
    wq = np.ascontiguousarray(np.asarray(wq, dtype=np.float32))
    wk = np.ascontiguousarray(np.asarray(wk, dtype=np.float32))
    wv = np.ascontiguousarray(np.asarray(wv, dtype=np.float32))
    n_cores = N_CORES
    nc = _get_nc(x.shape[0], x.shape[1], n_cores)
    in_maps = make_in_maps(x, wq, wk, wv, n_cores=n_cores)
    res = run_bass_kernel_spmd(nc, in_maps, list(range(n_cores)),
                               trace=trace, **spmd_kwargs)
    out = np.concatenate([res.results[c]["out"] for c in range(n_cores)], axis=0)
    return out, res


def kernel(x, wq, wk, wv):
    out, _ = run(x, wq, wk, wv, trace=False)
    return out
